# revision 11
# baseline (speedup 1.0000x reference)
"""Trainium2 Bass kernel for nn_ChannelSelfAttention.

Reference computation (per batch sample b):
    xt   = x[b].T                          # [C, L]
    q    = xt @ Wq.T + bq                  # [C, H]
    kv   = xt @ Wkv.T + bkv                # [C, 2H] -> k, v
    attn = (q * H**-0.5) @ k.T             # [C, C]  (no softmax)
    y    = attn @ v                        # [C, H]
    g    = mean(y, axis=-1)                # [C]
    out[b] = x[b] * g[None, :]             # [L, C]

No softmax -> everything after qkv is LINEAR, so the [C,C] attention
matrix is never materialized:

    g[c] = sum_h q''[c,h] * kvb[h]
    kvb[h] = sum_d k[d,h] * vbar[d],  vbar[d] = sum_h' v[d,h']

with scale/H folded into Wq/bq on the host (q'' = q * scale / H).

Sharding: data-parallel over B across 8 cores (4 samples per core);
weights replicated.  All HBM I/O is bf16 (host casts): 17.5 MiB per
core at the per-NC HBM/SDMA wall.

Bulk data (x, wT, out) lives on SBUF partitions 0-123 (124-wide tiles,
33 L-rows per partition per sample, plus a tiny 4-row remainder on
partitions 0-3): SDMA engine 15 - which intermittently runs ~20%
slower on trn2 and otherwise becomes the sole critical path, draining
its 1/16 byte share alone at the end - serves partitions {92-95,
124-127}, so this layout halves its load and hides the straggle at a
cost of 1/32 of aggregate DMA width.

Schedule: constants + remainder loads on the scalar HWDGE queue; bulk
loads on sync before any store issue; PE warm-up bridges the first
load wait at full-ish clock; PE emission software-pipelined across
samples; vbar reduces read qkv straight from PSUM and are emitted
ahead of the previous sample's gate multiplies; gate multiply chunked
~4x560KB per sample with stores issued per chunk; the last sample
splits its chunks across DVE and the idle GPSIMD engine so the final
stores enqueue earlier.
"""

import numpy as np
import ml_dtypes

import concourse.bass as bass
import concourse.mybir as mybir
import concourse.tile as tile
from concourse import bacc
from concourse.bass_utils import run_bass_kernel_spmd

B, L, C, H = 32, 4096, 256, 64
N_CORES = 8
B_LOC = B // N_CORES          # samples per core
P = 128                       # SBUF partitions (chain compute)
PB = 124                      # bulk-data partitions (avoid engine 15's upper 4)
JB = 33                       # L-rows per bulk partition per sample
RP = L - PB * JB              # remainder rows (4), on partitions 0-3
JA = 17                       # rows in bulk half A
JBH = JB - JA                 # rows in bulk half B (16)
GC = C // P                   # c-groups (2)
TH = 3 * H                    # 192 = q|k|v
BF16 = mybir.dt.bfloat16
F32 = mybir.dt.float32
SCALE = float(H) ** -0.5
BF = ml_dtypes.bfloat16
N_WARM = 14                   # PE warm-up junk matmuls (512 cols each)
WCOL = 512
# gate-multiply j-chunks per half: (j0, j1) within the half
CHUNKS = {0: [(0, 9), (9, JA)], 1: [(0, 8), (8, JBH)]}


def _emit(tc: "tile.TileContext", x_d, wT_d, bias_d, id_d, out_d) -> None:
    nc = tc.nc
    with (
        tc.tile_pool(name="singles", bufs=1) as singles,
        tc.tile_pool(name="xin", bufs=B_LOC) as xin,
        tc.tile_pool(name="xrem", bufs=B_LOC) as xremp,
        tc.tile_pool(name="xout", bufs=12) as xout,
        tc.tile_pool(name="orem", bufs=2) as oremp,
        tc.tile_pool(name="small", bufs=2) as small,
        tc.tile_pool(name="psW", bufs=1, space="PSUM") as psW,
        tc.tile_pool(name="psA", bufs=2, space="PSUM") as psA,
        tc.tile_pool(name="psA2", bufs=2, space="PSUM") as psA2,
        tc.tile_pool(name="psM", bufs=2, space="PSUM") as psM,
        tc.tile_pool(name="psG", bufs=1, space="PSUM") as psG,
    ):
        # ---- constants + remainder weights on the scalar (ACT HWDGE)
        # queue: tiny, lands early, steals nothing from the sync ring ----
        cb = singles.tile([1, P + TH], BF16)             # ones | (bq*s/H)|bkv
        nc.scalar.dma_start(out=cb, in_=bias_d[:])
        ones1 = cb[:, 0:P]
        bias_sb = cb[:, P : P + TH]
        ident = singles.tile([P, P], BF16)
        nc.scalar.dma_start(out=ident, in_=id_d[:])

        # ---- PE warm-up: one continuous run of wide junk matmuls so the
        # HAM clock-gate reaches full p-state before real work ----
        scratch = singles.tile([P, P + WCOL], BF16)
        nc.vector.memset(scratch, 0.0)
        psj = psW.tile([P, WCOL], F32, tag="warm", name="psj")
        for _ in range(N_WARM):
            nc.tensor.matmul(psj, lhsT=scratch[:, 0:P],
                             rhs=scratch[:, P : P + WCOL])

        # ---- source/dest APs: 124-partition bulk + 4-row remainder ----
        # NOTE: all bulk DRAM APs are built flat ([p stride, 124] x [1, N])
        # so each partition's rows stay ONE contiguous descriptor; slicing a
        # rearranged "p j c" view keeps the j dim unmerged and fragments the
        # DMA into 512B descriptors (measured 3x slowdown).
        def _flat(base, row0, nrows, width):
            return bass.AP(
                tensor=base.tensor,
                offset=base.offset + row0 * width,
                ap=[[JB * width, PB], [1, nrows * width]],
            )

        wbase = wT_d[:]
        w_srcs = [_flat(wbase, 0, JA, TH), _flat(wbase, JA, JBH, TH)]
        wR_src = wT_d[:][PB * JB : L].rearrange("(p j) h -> p (j h)", p=RP, j=1)
        x_srcs, xR_srcs, out_bases, outR_dsts = [], [], [], []
        for b in range(B_LOC):
            x_srcs.append([_flat(x_d[b], 0, JA, C), _flat(x_d[b], JA, JBH, C)])
            xR_srcs.append(
                x_d[b][PB * JB : L].rearrange("(p j) c -> p (j c)", p=RP, j=1)
            )
            out_bases.append(out_d[b])
            outR_dsts.append(
                out_d[b][PB * JB : L].rearrange("(p j) c -> p (j c)", p=RP, j=1)
            )

        # ---- bulk loads on the sync ring, in consumption order ----
        wts = [singles.tile([PB, JA * TH], BF16, name="wtA"),
               singles.tile([PB, JBH * TH], BF16, name="wtB")]
        xs = [[xin.tile([PB, JA * C], BF16, tag="xA", name=f"x_b{b}_A"),
               xin.tile([PB, JBH * C], BF16, tag="xB", name=f"x_b{b}_B")]
              for b in range(B_LOC)]
        for h in range(2):
            nc.sync.dma_start(out=wts[h], in_=w_srcs[h])
            nc.sync.dma_start(out=xs[0][h], in_=x_srcs[0][h])
        for b in range(1, B_LOC):
            for h in range(2):
                nc.sync.dma_start(out=xs[b][h], in_=x_srcs[b][h])
        # remainder rows ride the scalar queue (tiny; must not queue
        # behind 9.5MB of bulk loads on the sync FIFO)
        wtR = singles.tile([RP, TH], BF16, name="wtR")
        nc.scalar.dma_start(out=wtR, in_=wR_src)
        xrems = []
        for b in range(B_LOC):
            xrems.append(xremp.tile([RP, C], BF16, tag="xr", name=f"xr_b{b}"))
            nc.scalar.dma_start(out=xrems[b], in_=xR_srcs[b])

        def qkv_bias(b):
            """Open sample b's PSUM accumulation groups with the bias."""
            pq = [psA.tile([P, TH], F32, tag="qkv0", name=f"pq0_b{b}"),
                  psA2.tile([P, TH], F32, tag="qkv1", name=f"pq1_b{b}")]
            for g in range(GC):
                nc.tensor.matmul(
                    pq[g], lhsT=ones1, rhs=bias_sb, start=True, stop=False,
                )
            return pq

        def qkv_half(b, h, pq):
            """Chunk matmuls for bulk half h of sample b (x-stationary)."""
            nj = JA if h == 0 else JBH
            for j in range(nj):
                for g in range(GC):
                    nc.tensor.matmul(
                        pq[g],
                        lhsT=xs[b][h][:, j * C + g * P : j * C + (g + 1) * P],
                        rhs=wts[h][:, j * TH : (j + 1) * TH],
                        start=False, stop=False,
                    )

        def qkv_rem(b, pq):
            """Remainder rows close both accumulation groups."""
            for g in range(GC):
                nc.tensor.matmul(
                    pq[g], lhsT=xrems[b][:, g * P : (g + 1) * P], rhs=wtR,
                    start=False, stop=(g == GC - 1) or True,
                )

        def qkv_copy(b, pq):
            qkv_sb = small.tile([P, GC, TH], BF16, tag="qkv_sb")
            for g in range(GC):
                nc.scalar.copy(qkv_sb[:, g], pq[g])
            return qkv_sb

        def vbar_stage(b, pq):
            """vbar[d] = sum_h v[d,h], read straight from the qkv PSUM."""
            vbar_sb = small.tile([P, GC, 1], BF16, tag="vbar")
            with nc.allow_low_precision(reason="bf16 vbar feeds bf16 matmul"):
                for g in range(GC):
                    nc.vector.tensor_reduce(
                        out=vbar_sb[:, g], in_=pq[g][:, 2 * H : TH],
                        axis=mybir.AxisListType.X, op=mybir.AluOpType.add,
                    )
            return vbar_sb

        def chain_stage(b, qkv_sb, vbar_sb):
            """kvb + q^T -> one PSUM tile -> one copy -> g matmul."""
            ps_m = psM.tile([H, P + C], F32, tag="m")
            # kvb[h] (broadcast along 128 free cols): lhsT = k-section,
            # rhs = vbar as a stride-0 broadcast row
            for g in range(GC):
                vb_bc = bass.AP(
                    tensor=vbar_sb.tensor,
                    offset=vbar_sb.offset + g * vbar_sb.ap[1][0],
                    ap=[list(vbar_sb.ap[0]), [0, P]],
                )
                nc.tensor.matmul(
                    ps_m[:, 0:P], lhsT=qkv_sb[:, g, H : 2 * H], rhs=vb_bc,
                    start=(g == 0), stop=(g == GC - 1),
                )
            # q^T [64, 256] via PE matmul against the identity
            for g in range(GC):
                nc.tensor.matmul(
                    ps_m[:, P + g * P : P + (g + 1) * P],
                    lhsT=qkv_sb[:, g, 0:H], rhs=ident,
                )
            m_sb = small.tile([H, P + C], BF16, tag="m_sb")
            nc.scalar.copy(m_sb, ps_m)
            # g[c] = sum_h kvb[h] qT[h, c], landing on all 128 partitions
            ps_g = psG.tile([P, C], F32, tag="g")
            nc.tensor.matmul(ps_g, lhsT=m_sb[:, 0:P], rhs=m_sb[:, P : P + C])
            g_sb = small.tile([P, C], BF16, tag="g_sb")
            nc.scalar.copy(g_sb, ps_g)
            return g_sb

        def gate_rem(b, g_sb):
            """Remainder-row gate multiply (DVE) + store on sync."""
            o_r = oremp.tile([RP, C], BF16, tag="or", name=f"or_b{b}")
            g4 = bass.AP(tensor=g_sb.tensor, offset=g_sb.offset,
                         ap=[[g_sb.ap[0][0], RP], list(g_sb.ap[1])])
            nc.vector.tensor_tensor(
                out=o_r, in0=xrems[b], in1=g4, op=mybir.AluOpType.mult,
            )
            nc.sync.dma_start(out=outR_dsts[b], in_=o_r)

        def gate_store(b, h, ci, g_sb, eng):
            """~560KB gate multiply + store for chunk ci of half h."""
            j0, j1 = CHUNKS[h][ci]
            nj = j1 - j0
            base = 0 if h == 0 else JA
            g_bc = bass.AP(
                tensor=g_sb.tensor,
                offset=g_sb.offset,
                ap=[[g_sb.ap[0][0], PB], [0, nj], list(g_sb.ap[1])],
            )
            o_t = xout.tile([PB, 9 * C], BF16, tag="o", name=f"o_b{b}h{h}c{ci}")
            eng.tensor_tensor(
                out=o_t[:, 0 : nj * C].rearrange("p (j c) -> p j c", c=C),
                in0=xs[b][h][:, j0 * C : j1 * C]
                .rearrange("p (j c) -> p j c", c=C),
                in1=g_bc,
                op=mybir.AluOpType.mult,
            )
            nc.sync.dma_start(
                out=_flat(out_bases[b], base + j0, nj, C),
                in_=o_t[:, 0 : nj * C],
            )

        # ---- software-pipelined emission over samples ----
        pq = qkv_bias(0)
        qkv_half(0, 0, pq)
        qkv_half(0, 1, pq)
        qkv_rem(0, pq)
        cur_sb = qkv_copy(0, pq)
        cur_vb = vbar_stage(0, pq)
        for b in range(B_LOC):
            nxt_pq = None
            if b + 1 < B_LOC:
                nxt_pq = qkv_bias(b + 1)
                qkv_half(b + 1, 0, nxt_pq)
            g_sb = chain_stage(b, cur_sb, cur_vb)
            if b + 1 < B_LOC:
                qkv_half(b + 1, 1, nxt_pq)
                qkv_rem(b + 1, nxt_pq)
                cur_sb = qkv_copy(b + 1, nxt_pq)
                cur_vb = vbar_stage(b + 1, nxt_pq)
            gate_rem(b, g_sb)
            if b < B_LOC - 1:
                for h in range(2):
                    for ci in range(2):
                        gate_store(b, h, ci, g_sb, nc.vector)
            else:
                # last sample: DVE takes half B first (its stores lead in
                # the ring FIFO), the idle GPSIMD engine computes half A
                # concurrently so the final stores enqueue earlier
                for ci in range(2):
                    gate_store(b, 1, ci, g_sb, nc.vector)
                for ci in range(2):
                    gate_store(b, 0, ci, g_sb, nc.gpsimd)


def build():
    nc = bacc.Bacc(
        "TRN2", target_bir_lowering=False, debug=False, num_devices=N_CORES
    )
    x_d = nc.dram_tensor("x", [B_LOC, L, C], BF16, kind="ExternalInput")
    wT_d = nc.dram_tensor("wT", [L, TH], BF16, kind="ExternalInput")
    bias_d = nc.dram_tensor("bias", [1, P + TH], BF16, kind="ExternalInput")
    id_d = nc.dram_tensor("ident", [P, P], BF16, kind="ExternalInput")
    out_d = nc.dram_tensor("out", [B_LOC, L, C], BF16, kind="ExternalOutput")
    with tile.TileContext(nc) as tc:
        _emit(tc, x_d, wT_d, bias_d, id_d, out_d)
    nc.compile()
    return nc


_nc_cache = None


def _get_nc():
    global _nc_cache
    if _nc_cache is None:
        _nc_cache = build()
    return _nc_cache


def make_in_maps(x, Wq, bq, Wkv, bkv):
    x_bf = np.asarray(x, dtype=np.float32).astype(BF)
    qs = SCALE / H                      # fold attn scale AND mean-over-H into q
    wT = np.ascontiguousarray(
        np.concatenate(
            [np.asarray(Wq, np.float32) * qs, np.asarray(Wkv, np.float32)],
            axis=0,
        ).T.astype(BF)
    )
    bias = np.concatenate(
        [np.asarray(bq, np.float32) * qs, np.asarray(bkv, np.float32)]
    )[None].astype(BF)
    ident = np.eye(P, dtype=BF)
    cb = np.concatenate([np.ones((1, P), dtype=BF), bias], axis=1)
    return [
        {
            "x": np.ascontiguousarray(x_bf[i * B_LOC : (i + 1) * B_LOC]),
            "wT": wT,
            "bias": cb,
            "ident": ident,
        }
        for i in range(N_CORES)
    ]


def run(inputs, **spmd_kwargs):
    """Run on hardware; returns (full_output, BassKernelResults)."""
    nc = _get_nc()
    in_maps = make_in_maps(**inputs)
    res = run_bass_kernel_spmd(nc, in_maps, list(range(N_CORES)), **spmd_kwargs)
    out = np.concatenate([r["out"] for r in res.results], axis=0)
    return np.asarray(out).astype(np.float32), res


def kernel(**inputs) -> np.ndarray:
    out, _ = run(inputs)
    return out


# revision 12
# speedup vs baseline: 3.0293x; 3.0293x over previous
"""Trainium2 Bass kernel for nn_ChannelSelfAttention.

Reference computation (per batch sample b):
    xt   = x[b].T                          # [C, L]
    q    = xt @ Wq.T + bq                  # [C, H]
    kv   = xt @ Wkv.T + bkv                # [C, 2H] -> k, v
    attn = (q * H**-0.5) @ k.T             # [C, C]  (no softmax)
    y    = attn @ v                        # [C, H]
    g    = mean(y, axis=-1)                # [C]
    out[b] = x[b] * g[None, :]             # [L, C]

No softmax -> everything after qkv is LINEAR, so the [C,C] attention
matrix is never materialized:

    g[c] = sum_h q''[c,h] * kvb[h]
    kvb[h] = sum_d k[d,h] * vbar[d],  vbar[d] = sum_h' v[d,h']

with scale/H folded into Wq/bq on the host (q'' = q * scale / H).

Sharding: data-parallel over B across 8 cores (4 samples per core);
weights replicated.  All HBM I/O is bf16 (host casts): 17.5 MiB per
core, and the DMA stream runs at the per-NC HBM/SDMA wall, so the
schedule keeps the sync ring 100% fed and the tail short:

  - Constants ride the scalar (ACT HWDGE) queue; bulk x/wT/out on sync
    with 8KB/6KB descriptors (j=16 rows per partition per chunk; the
    DMA engine assignment splits descriptor blocks evenly, so counts
    must stay 128-divisible).
  - wT in two per-chunk tiles interleaved with sample 0's x halves so
    the first qkv matmuls start ~2us earlier.
  - Loads issue before any store so store-gate semaphore waits on the
    SP sequencer can't delay a load.
  - PE warm-up is one continuous ~6us run of wide junk matmuls so the
    clock p-state is ramped when the first real matmul issues.
  - PE emission is software-pipelined: qkv(b+1) half-0 between qkv(b)
    drain and sample b's tail matmuls, half-1 after.
  - vbar reduces read qkv directly from PSUM (no wait on the ACT copy)
    and are emitted ahead of the previous sample's gate multiplies so
    Tile's readiness scheduler never parks them behind 4.5us of DVE.
  - Per-sample tail chain: [ACT qkv copy || DVE vbar] -> PE (kvb via a
    stride-0 broadcast rhs of vbar, + q transposes, one shared PSUM
    tile) -> one ACT copy -> PE g matmul -> ACT g copy -> DVE gate
    multiplies, chunked 4x512KB with a store issued per chunk; the
    last sample splits its chunks across DVE and the idle GPSIMD
    engine so the final stores enqueue earlier.
"""

import numpy as np
import ml_dtypes

import concourse.bass as bass
import concourse.mybir as mybir
import concourse.tile as tile
from concourse import bacc
from concourse.bass_utils import run_bass_kernel_spmd

B, L, C, H = 32, 4096, 256, 64
N_CORES = 8
B_LOC = B // N_CORES          # samples per core
P = 128                       # SBUF partitions
JC = 16                       # L-rows per partition per chunk (8KB bf16 descs)
NCH = L // (P * JC)           # l-chunks per sample (2)
GC = C // P                   # c-groups (2)
TH = 3 * H                    # 192 = q|k|v
BF16 = mybir.dt.bfloat16
F32 = mybir.dt.float32
SCALE = float(H) ** -0.5
BF = ml_dtypes.bfloat16
N_WARM = 14                   # PE warm-up junk matmuls (512 cols each)
WCOL = 512


def _emit(tc: "tile.TileContext", x_d, wT_d, bias_d, id_d, out_d) -> None:
    nc = tc.nc
    with (
        tc.tile_pool(name="singles", bufs=1) as singles,
        tc.tile_pool(name="xin", bufs=B_LOC) as xin,
        tc.tile_pool(name="xout", bufs=12) as xout,
        tc.tile_pool(name="small", bufs=2) as small,
        tc.tile_pool(name="psW", bufs=1, space="PSUM") as psW,
        tc.tile_pool(name="psA", bufs=2, space="PSUM") as psA,
        tc.tile_pool(name="psA2", bufs=2, space="PSUM") as psA2,
        tc.tile_pool(name="psM", bufs=2, space="PSUM") as psM,
        tc.tile_pool(name="psG", bufs=1, space="PSUM") as psG,
    ):
        # ---- constants on the scalar (ACT HWDGE) queue: tiny, and the
        # SWDGE path's ~1us-per-DMA descriptor gen lands them too late ----
        cb = singles.tile([1, P + TH], BF16)             # ones | (bq*s/H)|bkv
        nc.scalar.dma_start(out=cb, in_=bias_d[:])
        ones1 = cb[:, 0:P]
        bias_sb = cb[:, P : P + TH]
        ident = singles.tile([P, P], BF16)
        nc.scalar.dma_start(out=ident, in_=id_d[:])

        # ---- PE warm-up: one continuous run of wide junk matmuls so the
        # HAM clock-gate reaches full p-state before real work ----
        scratch = singles.tile([P, P + WCOL], BF16)
        nc.vector.memset(scratch, 0.0)
        psj = psW.tile([P, WCOL], F32, tag="warm", name="psj")
        for _ in range(N_WARM):
            nc.tensor.matmul(psj, lhsT=scratch[:, 0:P],
                             rhs=scratch[:, P : P + WCOL])

        # ---- bulk loads on the sync ring, in consumption order ----
        wT_src = wT_d[:].rearrange("(n p j) h -> p n (j h)", p=P, j=JC)
        x_srcs = [x_d[b].rearrange("(n p j) c -> n p (j c)", p=P, j=JC)
                  for b in range(B_LOC)]
        out_dsts = [out_d[b].rearrange("(n p j) c -> n p (j c)", p=P, j=JC)
                    for b in range(B_LOC)]
        # wT in two per-chunk tiles interleaved with sample 0's x halves:
        # qkv(0) half-0 can start right after 1.75MB instead of 2.5MB.
        wts = [singles.tile([P, JC * TH], BF16, name=f"wt{h}")
               for h in range(NCH)]
        xs = [[xin.tile([P, JC * C], BF16, tag=f"x{h}", name=f"x_b{b}_h{h}")
               for h in range(NCH)] for b in range(B_LOC)]
        for h in range(NCH):
            nc.sync.dma_start(
                out=wts[h],
                in_=wT_src[:, h : h + 1].rearrange("p n x -> p (n x)"),
            )
            nc.sync.dma_start(
                out=xs[0][h],
                in_=x_srcs[0][h : h + 1].rearrange("n p x -> p (n x)"),
            )
        for b in range(1, B_LOC):
            for h in range(NCH):
                nc.sync.dma_start(
                    out=xs[b][h],
                    in_=x_srcs[b][h : h + 1].rearrange("n p x -> p (n x)"),
                )

        def qkv_bias(b):
            """Open sample b's PSUM accumulation groups with the bias."""
            pq = [psA.tile([P, TH], F32, tag="qkv0", name=f"pq0_b{b}"),
                  psA2.tile([P, TH], F32, tag="qkv1", name=f"pq1_b{b}")]
            for g in range(GC):
                nc.tensor.matmul(
                    pq[g], lhsT=ones1, rhs=bias_sb, start=True, stop=False,
                )
            return pq

        def qkv_half(b, h, pq):
            """Chunk matmuls for l-chunk h of sample b (x-stationary)."""
            for j in range(JC):
                for g in range(GC):
                    nc.tensor.matmul(
                        pq[g],
                        lhsT=xs[b][h][:, j * C + g * P : j * C + (g + 1) * P],
                        rhs=wts[h][:, j * TH : (j + 1) * TH],
                        start=False,
                        stop=(h == NCH - 1 and j == JC - 1),
                    )

        def qkv_copy(b, pq):
            qkv_sb = small.tile([P, GC, TH], BF16, tag="qkv_sb")
            for g in range(GC):
                nc.scalar.copy(qkv_sb[:, g], pq[g])
            return qkv_sb

        def vbar_stage(b, pq):
            """vbar[d] = sum_h v[d,h], read straight from the qkv PSUM."""
            vbar_sb = small.tile([P, GC, 1], BF16, tag="vbar")
            with nc.allow_low_precision(reason="bf16 vbar feeds bf16 matmul"):
                for g in range(GC):
                    nc.vector.tensor_reduce(
                        out=vbar_sb[:, g], in_=pq[g][:, 2 * H : TH],
                        axis=mybir.AxisListType.X, op=mybir.AluOpType.add,
                    )
            return vbar_sb

        def chain_stage(b, qkv_sb, vbar_sb):
            """kvb + q^T -> one PSUM tile -> one copy -> g matmul."""
            ps_m = psM.tile([H, P + C], F32, tag="m")
            # kvb[h] (broadcast along 128 free cols): lhsT = k-section,
            # rhs = vbar as a stride-0 broadcast row
            for g in range(GC):
                vb_bc = bass.AP(
                    tensor=vbar_sb.tensor,
                    offset=vbar_sb.offset + g * vbar_sb.ap[1][0],
                    ap=[list(vbar_sb.ap[0]), [0, P]],
                )
                nc.tensor.matmul(
                    ps_m[:, 0:P], lhsT=qkv_sb[:, g, H : 2 * H], rhs=vb_bc,
                    start=(g == 0), stop=(g == GC - 1),
                )
            # q^T [64, 256] via PE matmul against the identity
            for g in range(GC):
                nc.tensor.matmul(
                    ps_m[:, P + g * P : P + (g + 1) * P],
                    lhsT=qkv_sb[:, g, 0:H], rhs=ident,
                )
            m_sb = small.tile([H, P + C], BF16, tag="m_sb")
            nc.scalar.copy(m_sb, ps_m)
            # g[c] = sum_h kvb[h] qT[h, c], landing on all 128 partitions
            ps_g = psG.tile([P, C], F32, tag="g")
            nc.tensor.matmul(ps_g, lhsT=m_sb[:, 0:P], rhs=m_sb[:, P : P + C])
            g_sb = small.tile([P, C], BF16, tag="g_sb")
            nc.scalar.copy(g_sb, ps_g)
            return g_sb

        def gate_store(b, h, jh, g_sb, eng):
            """512KB gate multiply + store for quarter (h, jh) of sample b."""
            g_bc = bass.AP(
                tensor=g_sb.tensor,
                offset=g_sb.offset,
                ap=[list(g_sb.ap[0]), [0, JC // 2], list(g_sb.ap[1])],
            )
            half = JC // 2 * C
            o_t = xout.tile([P, half], BF16, tag="o", name=f"o_b{b}_h{h}j{jh}")
            eng.tensor_tensor(
                out=o_t.rearrange("p (j c) -> p j c", c=C),
                in0=xs[b][h][:, jh * half : (jh + 1) * half]
                .rearrange("p (j c) -> p j c", c=C),
                in1=g_bc,
                op=mybir.AluOpType.mult,
            )
            nc.sync.dma_start(
                out=bass.AP(
                    tensor=out_dsts[b][h].tensor,
                    offset=out_dsts[b][h].offset + jh * half,
                    ap=[list(out_dsts[b][h].ap[0]), [1, half]],
                ),
                in_=o_t,
            )

        # ---- software-pipelined emission over samples ----
        pq = qkv_bias(0)
        qkv_half(0, 0, pq)
        qkv_half(0, 1, pq)
        cur_sb = qkv_copy(0, pq)
        cur_vb = vbar_stage(0, pq)
        for b in range(B_LOC):
            nxt_pq = None
            if b + 1 < B_LOC:
                nxt_pq = qkv_bias(b + 1)
                qkv_half(b + 1, 0, nxt_pq)
            g_sb = chain_stage(b, cur_sb, cur_vb)
            if b + 1 < B_LOC:
                qkv_half(b + 1, 1, nxt_pq)
                cur_sb = qkv_copy(b + 1, nxt_pq)
                cur_vb = vbar_stage(b + 1, nxt_pq)
            if b < B_LOC - 1:
                for h in range(NCH):
                    for jh in range(2):
                        gate_store(b, h, jh, g_sb, nc.vector)
            else:
                # last sample: DVE takes half 1 first (its stores lead in
                # the ring FIFO), the idle GPSIMD engine computes half 0
                # concurrently so the final stores enqueue ~2.5us earlier
                for jh in range(2):
                    gate_store(b, 1, jh, g_sb, nc.vector)
                for jh in range(2):
                    gate_store(b, 0, jh, g_sb, nc.gpsimd)


def build():
    nc = bacc.Bacc(
        "TRN2", target_bir_lowering=False, debug=False, num_devices=N_CORES
    )
    x_d = nc.dram_tensor("x", [B_LOC, L, C], BF16, kind="ExternalInput")
    wT_d = nc.dram_tensor("wT", [L, TH], BF16, kind="ExternalInput")
    bias_d = nc.dram_tensor("bias", [1, P + TH], BF16, kind="ExternalInput")
    id_d = nc.dram_tensor("ident", [P, P], BF16, kind="ExternalInput")
    out_d = nc.dram_tensor("out", [B_LOC, L, C], BF16, kind="ExternalOutput")
    with tile.TileContext(nc) as tc:
        _emit(tc, x_d, wT_d, bias_d, id_d, out_d)
    nc.compile()
    return nc


_nc_cache = None


def _get_nc():
    global _nc_cache
    if _nc_cache is None:
        _nc_cache = build()
    return _nc_cache


def make_in_maps(x, Wq, bq, Wkv, bkv):
    x_bf = np.asarray(x, dtype=np.float32).astype(BF)
    qs = SCALE / H                      # fold attn scale AND mean-over-H into q
    wT = np.ascontiguousarray(
        np.concatenate(
            [np.asarray(Wq, np.float32) * qs, np.asarray(Wkv, np.float32)],
            axis=0,
        ).T.astype(BF)
    )
    bias = np.concatenate(
        [np.asarray(bq, np.float32) * qs, np.asarray(bkv, np.float32)]
    )[None].astype(BF)
    ident = np.eye(P, dtype=BF)
    cb = np.concatenate([np.ones((1, P), dtype=BF), bias], axis=1)
    return [
        {
            "x": np.ascontiguousarray(x_bf[i * B_LOC : (i + 1) * B_LOC]),
            "wT": wT,
            "bias": cb,
            "ident": ident,
        }
        for i in range(N_CORES)
    ]


def run(inputs, **spmd_kwargs):
    """Run on hardware; returns (full_output, BassKernelResults)."""
    nc = _get_nc()
    in_maps = make_in_maps(**inputs)
    res = run_bass_kernel_spmd(nc, in_maps, list(range(N_CORES)), **spmd_kwargs)
    out = np.concatenate([r["out"] for r in res.results], axis=0)
    return np.asarray(out).astype(np.float32), res


def kernel(**inputs) -> np.ndarray:
    out, _ = run(inputs)
    return out
